# revision 9
# baseline (speedup 1.0000x reference)
"""Trainium2 Bass kernel for nn_LoraInjectedLinear (moe_routing).

Computation (per chunk b of 16):
    idx_b  = lora_id[b] // 4, active_b = lora_id[b] >= 0
    out[b] = x[b] @ W.T + active_b * SCALE * (x[b] @ Wd[idx_b].T) @ Wu[idx_b].T

Strategy:
  - Host folds the rank-4 LoRA pair into a per-chunk fused weight:
        W_aug[b] = W + active_b * SCALE * Wu[idx_b] @ Wd[idx_b]
    and pre-transposes both the fused weight ([d, o] layout) and x
    ([d, t] layout) so the device kernel is a pure batched GEMM with the
    contraction dim on SBUF partitions (no on-device transposes).
  - Data parallel across 8 NeuronCores: 2 chunks per core.
  - All device I/O in fp16 (x, fused W, out). PE streams fp16 at the
    same 1 cycle/row as fp32r but LDWEIGHTS takes half the rows and
    HBM traffic halves; PSUM accumulates in fp32, host casts back.
  - First block runs in o-chunk passes against column-piece weight DMAs
    so early compute only waits on W columns 0:512, not all 1280.
"""

import os

import numpy as np

G = 16  # chunks
T = 4096  # tokens per chunk
D_IN = 1280
D_OUT = 1280
RANK = 4
LORA_STRIDE = 4
SCALE = 1.0

N_CORES = 8
CPC = G // N_CORES  # chunks per core = 2

P = 128
D_TILES = D_IN // P  # 10
T_BLK = 512  # tokens per x DMA block
T_SUB = T_BLK // P  # 4 subtiles of 128 tokens
N_BLKS = T // T_BLK  # 8 blocks per chunk
O_CHUNKS = [(0, 512), (512, 512), (1024, 256)]  # N-slices of D_OUT

_NC = None


def _build():
    global _NC
    if _NC is not None:
        return _NC

    import concourse.mybir as mybir
    from concourse import bacc
    from concourse.tile import TileContext

    f16 = mybir.dt.float16
    f32 = mybir.dt.float32

    nc = bacc.Bacc()
    xT = nc.declare_dram_parameter("xT", [CPC, D_IN, T], f16, isOutput=False)
    wT = nc.declare_dram_parameter("wT", [CPC, D_IN, D_OUT], f16, isOutput=False)
    out = nc.declare_dram_parameter("out", [CPC, T, D_OUT], f16, isOutput=True)

    with TileContext(nc) as tc:
        with (
            tc.tile_pool(name="wpool", bufs=2 * D_TILES) as wpool,
            tc.tile_pool(name="xpool", bufs=2) as xpool,
            tc.tile_pool(name="opool", bufs=5) as opool,
            tc.tile_pool(name="pspool", bufs=8, space="PSUM") as pspool,
        ):
            # Small PE warm-up on a zeroed scratch tile: if the PE boots
            # before the first data DMAs land, these spin the DVFS ramp
            # on throwaway work; if data is already there they cost ~1us.
            warm = wpool.tile([P, P], f16, name="warm", tag="warm")
            nc.vector.memset(warm[:], 0)
            ps_warm = pspool.tile([P, 512], f32, name="ps_warm", tag="ps")
            for _ in range(6):
                nc.tensor.matmul(
                    ps_warm[:, :P],
                    lhsT=warm[:],
                    rhs=warm[:],
                    start=True,
                    stop=True,
                )

            # First x block, sliced per d-tile on the SYNC ring: 1 KB
            # descriptor runs (>=512 B keeps full DMA rate) and the
            # first block's waves depend only on slice n, so compute
            # tracks the arriving stream at d-tile granularity.
            xt0 = xpool.tile([P, D_TILES, T_BLK], f16)
            xsrc0 = xT.ap()[0, :, 0:T_BLK].rearrange("(n p) t -> p n t", p=P)
            for n in range(D_TILES):
                nc.sync.dma_start(xt0[:, n, :], xsrc0[:, n, :])

            # Weights split across the ACT and SYNC HWDGE rings (both
            # hardware-DGE; the GpSimd ring is software-DGE and far too
            # slow). Each d-tile loads in three column pieces, issued
            # o-chunk-major, so the first block's o-chunk passes only
            # wait on the piece they actually read.
            def load_weights(c):
                wsrc = wT.ap()[c].rearrange("(n p) o -> p n o", p=P)
                row = [
                    wpool.tile([P, D_OUT], f16, name=f"w_{c}_{n}", tag="wt")
                    for n in range(D_TILES)
                ]
                for o0, ow in O_CHUNKS:
                    for n in range(D_TILES):
                        eng = nc.scalar if n % 2 == 0 else nc.sync
                        eng.dma_start(
                            row[n][:, o0 : o0 + ow], wsrc[:, n, o0 : o0 + ow]
                        )
                return row

            wts = {0: load_weights(0)}

            def copy_chunk(ot, ps, oi):
                o0, ow = O_CHUNKS[oi]
                if oi == 1:
                    nc.vector.tensor_copy(ot[:, o0 : o0 + ow], ps[:, :ow])
                else:
                    nc.scalar.copy(ot[:, o0 : o0 + ow], ps[:, :ow])

            def store(ot, c, j, s, oi=None):
                dst = out.ap()[c, (j * T_SUB + s) * P : (j * T_SUB + s + 1) * P, :]
                if oi is None:
                    nc.scalar.dma_start(dst, ot[:])
                else:
                    o0, ow = O_CHUNKS[oi]
                    nc.scalar.dma_start(
                        dst[:, o0 : o0 + ow], ot[:, o0 : o0 + ow]
                    )

            def mm(ps, xt, n, s, oi, start, stop):
                o0, ow = O_CHUNKS[oi]
                nc.tensor.matmul(
                    ps[:, :ow],
                    lhsT=xt[:, n, s * P : (s + 1) * P],
                    rhs=wts_cur[n][:, o0 : o0 + ow],
                    start=start,
                    stop=stop,
                )

            for c in range(CPC):
                wts_cur = wts[c]
                for j in range(N_BLKS):
                    if c == 0 and j == 0:
                        xt = xt0
                    else:
                        xt = xpool.tile([P, D_TILES, T_BLK], f16)
                        xsrc = xT.ap()[c, :, j * T_BLK : (j + 1) * T_BLK].rearrange(
                            "(n p) t -> p n t", p=P
                        )
                        nc.sync.dma_start(xt[:], xsrc)
                    if c == 0 and j == 2:
                        wts[1] = load_weights(1)

                    last = c == CPC - 1 and j == N_BLKS - 1
                    if c == 0 and j == 0:
                        # Ramp in o-chunk passes (d-tile outer within
                        # each pass): pass oi needs only W piece oi of
                        # each d-tile plus x slice n.
                        ots = [
                            opool.tile([P, D_OUT], f16, name=f"ot_r{s}", tag="ot")
                            for s in range(T_SUB)
                        ]
                        pss = {}
                        for oi in range(3):
                            for s in range(T_SUB):
                                pss[(oi, s)] = pspool.tile(
                                    [P, 512], f32, name=f"ps_r{oi}_{s}", tag="ps"
                                )
                            for n in range(D_TILES):
                                for s in range(T_SUB):
                                    mm(pss[(oi, s)], xt, n, s, oi,
                                       n == 0, n == D_TILES - 1)
                            for s in range(T_SUB):
                                copy_chunk(ots[s], pss[(oi, s)], oi)
                        for s in range(T_SUB):
                            store(ots[s], c, j, s)
                    else:
                        for s in range(T_SUB):
                            ot = opool.tile([P, D_OUT], f16)
                            ps_row = [
                                pspool.tile([P, 512], f32, name="ps", tag="ps")
                                for _ in O_CHUNKS
                            ]
                            if last and s == T_SUB - 1:
                                # final subtile: sequential o-chunk groups
                                # so copies/stores overlap the tail mms
                                for oi in range(3):
                                    for n in range(D_TILES):
                                        mm(ps_row[oi], xt, n, s, oi,
                                           n == 0, n == D_TILES - 1)
                                    copy_chunk(ot, ps_row[oi], oi)
                                    store(ot, c, j, s, oi=oi)
                            else:
                                for n in range(D_TILES):
                                    for oi in range(3):
                                        mm(ps_row[oi], xt, n, s, oi,
                                           n == 0, n == D_TILES - 1)
                                for oi in range(3):
                                    copy_chunk(ot, ps_row[oi], oi)
                                store(ot, c, j, s)
    nc.finalize()
    _NC = nc
    return nc


def _host_prep(x, lora_id, W, Wd, Wu):
    x = np.asarray(x, dtype=np.float32)
    lora_id = np.asarray(lora_id)
    W = np.asarray(W, dtype=np.float32)
    Wd = np.asarray(Wd, dtype=np.float32)
    Wu = np.asarray(Wu, dtype=np.float32)

    idx = lora_id.astype(np.int64) // LORA_STRIDE
    active = lora_id >= 0
    safe_idx = np.where(active, idx, 0)

    WT = np.ascontiguousarray(W.T)  # [d, o]
    waugT = np.empty((G, D_IN, D_OUT), dtype=np.float16)
    for b in range(G):
        if active[b]:
            i = int(safe_idx[b])
            # (Wu[i] @ Wd[i]).T = Wd[i].T @ Wu[i].T : [d, o]
            waugT[b] = WT + SCALE * (Wd[i].T @ Wu[i].T)
        else:
            waugT[b] = WT

    # [G, d, t] — contraction dim first so SBUF tiles need no transpose
    xT = np.ascontiguousarray(x.transpose(0, 2, 1)).astype(np.float16)
    return xT, waugT


def kernel(x, lora_id, W, Wd, Wu):
    from concourse.bass_utils import run_bass_kernel_spmd

    xT, waugT = _host_prep(x, lora_id, W, Wd, Wu)

    nc = _build()
    in_maps = [
        {"xT": xT[k * CPC : (k + 1) * CPC], "wT": waugT[k * CPC : (k + 1) * CPC]}
        for k in range(N_CORES)
    ]
    trace = bool(os.environ.get("KERNEL_PROFILE"))
    kwargs = {}
    if trace and os.environ.get("KERNEL_PROFILE_DIR"):
        kwargs["tmpdir"] = os.environ["KERNEL_PROFILE_DIR"]
    res = run_bass_kernel_spmd(nc, in_maps, list(range(N_CORES)), trace=trace, **kwargs)
    if trace:
        kernel.last_results = res
        print(f"HW exec time: {res.exec_time_ns} ns")
    return np.concatenate(
        [res.results[k]["out"] for k in range(N_CORES)], axis=0
    ).astype(np.float32)
